# revision 1
# baseline (speedup 1.0000x reference)
"""MoE routing kernel for Trainium2 (8 NeuronCores, expert-parallel).

Model (per layer l in 0..L-1):
    w = softmax(top-k masked |x @ protos[l].T|)          # [N, E] routing
    h = relu(x @ W1[l,e]); y = sum_e w[:,e] * (h @ W2[l,e])
    x = x + y

Sharding: expert-parallel - core c owns expert c for both layers (E == 8 ==
n_cores).  Every core computes the routing for all tokens (cheap), runs its
expert's FFN over all tokens in a feature-major ("transposed") layout so the
weights load untransposed as the stationary matmul operand, scales by its
routing column, and the weighted partials are summed with an on-device
AllReduce.  Core 0 additionally folds the residual x into its partial, so the
AllReduce output IS the next layer's input.  A second AllReduce after layer 2
produces the final output on every core.

The kernel is built once and cached at module level; repeated kernel() calls
reuse the compiled executable.
"""

import numpy as np

import concourse.bacc as bacc
import concourse.mybir as mybir
from concourse import tile
from concourse.masks import make_identity

P = 128


def build_moe(
    nc,
    D=1024,
    F=2048,
    NTOK=2048,
    E=8,
    L=2,
    K=2,
    BLK=256,
    ffn_f32r=True,
    nsplit=4,
):
    """Emit the SPMD MoE program into Bass instance `nc`."""
    DS = D // P       # D-slices (k-tiles for W1 / m-tiles for W2 output)
    FS = F // P       # F-slices
    NBLK = NTOK // BLK
    TT = BLK // P     # token-tiles per block
    f32 = mybir.dt.float32
    f32r = mybir.dt.float32r
    # fp32r operands must be explicitly rounded by their producer; weights are
    # rounded in place after load (bitcast view), x gets a rounded copy, and
    # the relu writes f32r directly.
    ffd = f32r if ffn_f32r else f32

    xT = nc.dram_tensor("xT", [D, NTOK], f32, kind="ExternalInput")
    prot = nc.dram_tensor("prot", [L, D, E], ffd, kind="ExternalInput")
    w1 = nc.dram_tensor("w1", [L, D, F], ffd, kind="ExternalInput")
    w2 = nc.dram_tensor("w2", [L, F, D], ffd, kind="ExternalInput")
    alpha = nc.dram_tensor("alpha", [P, 1], f32, kind="ExternalInput")
    onehot = nc.dram_tensor("onehot", [P, E], f32, kind="ExternalInput")
    yout = nc.dram_tensor("yout", [D, NTOK], f32, kind="ExternalOutput")

    with tile.TileContext(nc) as tc:
        with (
            tc.tile_pool(name="const", bufs=1) as constp,
            tc.tile_pool(name="wpool", bufs=1) as wpool,
            tc.tile_pool(name="xpool", bufs=2) as xpool,
            tc.tile_pool(name="route", bufs=2) as routep,
            tc.tile_pool(name="hpool", bufs=1) as hpool,
            tc.tile_pool(name="evict", bufs=3) as evpool,
            tc.tile_pool(name="wbcp", bufs=2) as wbcp,
            tc.tile_pool(name="psmisc", bufs=2, space="PSUM") as psmisc,
            tc.tile_pool(name="psh", bufs=2, space="PSUM") as psh,
            tc.tile_pool(name="psy", bufs=4, space="PSUM") as psy,
            tc.tile_pool(name="dram", bufs=2, space="DRAM") as dramp,
        ):
            ident = constp.tile([P, P], f32)
            make_identity(nc, ident)
            ones_row = constp.tile([1, P], f32)
            nc.vector.memset(ones_row, 1.0)
            alpha_sb = constp.tile([P, 1], f32)
            nc.sync.dma_start(alpha_sb[:], alpha.ap()[:])
            oh_sb = constp.tile([P, E], f32)
            nc.sync.dma_start(oh_sb[:], onehot.ap()[:])

            NH = NTOK // nsplit  # tokens per AR slice
            HBLK = NH // BLK     # blocks per slice
            assert HBLK >= 1
            xsrc_halves = [xT.ap()[:, h * NH : (h + 1) * NH] for h in range(nsplit)]
            for l in range(L):
                ypart = [
                    dramp.tile([D, NH], f32, tag=f"ypart{h}", name=f"ypart{h}_{l}")
                    for h in range(nsplit)
                ]
                ysum = [
                    dramp.tile([D, NH], f32, tag=f"ysum{h}", name=f"ysum{h}_{l}")
                    for h in range(nsplit)
                ]

                prot_sb = wpool.tile([P, DS, E], ffd, tag="prot")
                nc.sync.dma_start(
                    prot_sb[:], prot.ap()[l].rearrange("(t p) e -> p t e", p=P)
                )
                w1_sb = wpool.tile([P, DS, F], ffd, tag="w1")
                for ds in range(DS):
                    nc.sync.dma_start(
                        w1_sb[:, ds, :], w1.ap()[l, ds * P : (ds + 1) * P, :]
                    )
                w2_sb = wpool.tile([P, FS, D], ffd, tag="w2")
                for fs in range(FS):
                    nc.sync.dma_start(
                        w2_sb[:, fs, :], w2.ap()[l, fs * P : (fs + 1) * P, :]
                    )

                for nb in range(NBLK):
                    half = nb // HBLK
                    c0 = nb * BLK - half * NH  # col offset within the half
                    xsrc = xsrc_halves[half]
                    xb = xpool.tile([P, DS, BLK], f32, tag="xb")
                    for ds in range(DS):
                        nc.sync.dma_start(
                            xb[:, ds, :], xsrc[ds * P : (ds + 1) * P, c0 : c0 + BLK]
                        )
                    if ffn_f32r:
                        xbr = xpool.tile([P, DS, BLK], f32r, tag="xbr")
                        nc.vector.tensor_copy(xbr[:], xb[:])
                    else:
                        xbr = xb

                    # ---- routing: w column for this core's expert ----
                    ps_s = psmisc.tile([E, BLK], f32, tag="psm")
                    for ds in range(DS):
                        nc.tensor.matmul(
                            ps_s[:],
                            prot_sb[:, ds, :],
                            xbr[:, ds, :],
                            start=(ds == 0),
                            stop=(ds == DS - 1),
                        )
                    s_abs = routep.tile([E, BLK], f32, tag="sabs")
                    nc.scalar.activation(
                        s_abs[:], ps_s[:], mybir.ActivationFunctionType.Abs
                    )
                    s_tok = routep.tile([P, TT, E], f32, tag="stok")
                    for tt in range(TT):
                        ps_t = psmisc.tile([P, E], f32, tag="psm")
                        nc.tensor.transpose(
                            ps_t[:], s_abs[:, tt * P : (tt + 1) * P], ident[:E, :E]
                        )
                        nc.scalar.copy(s_tok[:, tt, :], ps_t[:])
                    srt = routep.tile([P, TT, E], f32, tag="srt")
                    for tt in range(TT):
                        nc.vector.max(srt[:, tt, :], s_tok[:, tt, :])
                    shif = routep.tile([P, TT, E], f32, tag="shif")
                    nc.vector.tensor_tensor(
                        out=shif[:],
                        in0=s_tok[:],
                        in1=srt[:, :, 0:1].to_broadcast([P, TT, E]),
                        op=mybir.AluOpType.subtract,
                    )
                    ex = routep.tile([P, TT, E], f32, tag="ex")
                    nc.scalar.activation(
                        ex[:], shif[:], mybir.ActivationFunctionType.Exp
                    )
                    mask = routep.tile([P, TT, E], f32, tag="mask")
                    nc.vector.tensor_tensor(
                        out=mask[:],
                        in0=s_tok[:],
                        in1=srt[:, :, K - 1 : K].to_broadcast([P, TT, E]),
                        op=mybir.AluOpType.is_ge,
                    )
                    nc.vector.tensor_tensor(
                        out=ex[:], in0=ex[:], in1=mask[:], op=mybir.AluOpType.mult
                    )
                    den = routep.tile([P, TT, 1], f32, tag="den")
                    nc.vector.reduce_sum(den[:], ex[:], axis=mybir.AxisListType.X)
                    rec = routep.tile([P, TT, 1], f32, tag="rec")
                    nc.vector.reciprocal(rec[:], den[:])
                    wtok = routep.tile([P, TT, E], f32, tag="wtok")
                    nc.vector.tensor_tensor(
                        out=wtok[:],
                        in0=ex[:],
                        in1=rec[:].to_broadcast([P, TT, E]),
                        op=mybir.AluOpType.mult,
                    )
                    # select this core's expert column (one-hot dot), token-major
                    wsel_g = routep.tile([P, TT, E], f32, tag="wselg")
                    nc.vector.tensor_tensor(
                        out=wsel_g[:],
                        in0=wtok[:],
                        in1=oh_sb[:].rearrange("p (t e) -> p t e", t=1).to_broadcast([P, TT, E]),
                        op=mybir.AluOpType.mult,
                    )
                    wsel = routep.tile([P, TT], f32, tag="wsel")
                    nc.vector.reduce_sum(
                        wsel[:].rearrange("p (t o) -> p t o", o=1),
                        wsel_g[:],
                        axis=mybir.AxisListType.X,
                    )
                    # transpose [P tok, TT] -> [TT, P]; flatten to a row; bcast
                    ps_w = psmisc.tile([TT, P], f32, tag="psm")
                    nc.tensor.transpose(ps_w[:], wsel[:], ident[:])
                    wrow4 = routep.tile([TT, P], f32, tag="wrow4")
                    nc.scalar.copy(wrow4[:], ps_w[:])
                    wrow = routep.tile([1, BLK], f32, tag="wrow")
                    nc.sync.dma_start(
                        wrow[:].rearrange("o (t p) -> o t p", t=TT), wrow4[:]
                    )
                    ps_b = psmisc.tile([P, BLK], f32, tag="psm")
                    nc.tensor.matmul(
                        ps_b[:], ones_row[:], wrow[:], start=True, stop=True
                    )
                    wbc = wbcp.tile([P, BLK], f32, tag="wbc")
                    nc.scalar.copy(wbc[:], ps_b[:])

                    # ---- FFN over this block ----
                    h_all = hpool.tile([P, FS, BLK], ffd, tag="h")
                    for fs in range(FS):
                        ps_h = psh.tile([P, BLK], f32, tag="psh")
                        for ds in range(DS):
                            nc.tensor.matmul(
                                ps_h[:],
                                w1_sb[:, ds, fs * P : (fs + 1) * P],
                                xbr[:, ds, :],
                                start=(ds == 0),
                                stop=(ds == DS - 1),
                            )
                        nc.scalar.activation(
                            h_all[:, fs, :], ps_h[:],
                            mybir.ActivationFunctionType.Relu,
                        )
                    for ds in range(DS):
                        ps_y = psy.tile([P, BLK], f32, tag="psy")
                        for fs in range(FS):
                            nc.tensor.matmul(
                                ps_y[:],
                                w2_sb[:, fs, ds * P : (ds + 1) * P],
                                h_all[:, fs, :],
                                start=(fs == 0),
                                stop=(fs == FS - 1),
                            )
                        yev = evpool.tile([P, BLK], f32, tag="yev")
                        nc.vector.tensor_tensor(
                            out=yev[:],
                            in0=ps_y[:],
                            in1=wbc[:],
                            op=mybir.AluOpType.mult,
                        )
                        nc.vector.scalar_tensor_tensor(
                            out=yev[:],
                            in0=xb[:, ds, :],
                            scalar=alpha_sb[:, 0:1],
                            in1=yev[:],
                            op0=mybir.AluOpType.mult,
                            op1=mybir.AluOpType.add,
                        )
                        nc.sync.dma_start(
                            ypart[half][ds * P : (ds + 1) * P, c0 : c0 + BLK],
                            yev[:],
                        )

                    if nb % HBLK == HBLK - 1:
                        nc.gpsimd.collective_compute(
                            "AllReduce",
                            mybir.AluOpType.add,
                            replica_groups=[list(range(E))],
                            ins=[ypart[half][:]],
                            outs=[ysum[half][:]],
                        )
                xsrc_halves = list(ysum)

            for h in range(nsplit):
                nc.sync.dma_start(
                    yout.ap()[:, h * NH : (h + 1) * NH], xsrc_halves[h][:]
                )
    return nc


_CACHE = {}


def _get_compiled():
    if "nc" not in _CACHE:
        nc = bacc.Bacc("TRN2", target_bir_lowering=False, debug=False, num_devices=8)
        build_moe(nc)
        nc.compile()
        _CACHE["nc"] = nc
    return _CACHE["nc"]


def kernel(x, protos, W1, W2, k):
    assert int(k) == 2
    B, S, Dx = x.shape
    L, E, D, F = W1.shape[0], W1.shape[1], W1.shape[2], W1.shape[3]
    N = B * S
    assert (B, S, Dx, L, E, D, F) == (2, 1024, 1024, 2, 8, 1024, 2048)

    nc = _get_compiled()

    xT = np.ascontiguousarray(np.asarray(x, dtype=np.float32).reshape(N, D).T)
    protT = np.ascontiguousarray(
        np.asarray(protos, dtype=np.float32).transpose(0, 2, 1)
    )
    W1 = np.asarray(W1, dtype=np.float32)
    W2 = np.asarray(W2, dtype=np.float32)

    in_maps = []
    for c in range(8):
        alpha = np.full((P, 1), 1.0 if c == 0 else 0.0, dtype=np.float32)
        oh = np.zeros((P, E), dtype=np.float32)
        oh[:, c] = 1.0
        in_maps.append(
            {
                "xT": xT,
                "prot": protT,
                "w1": np.ascontiguousarray(W1[:, c]),
                "w2": np.ascontiguousarray(W2[:, c]),
                "alpha": alpha,
                "onehot": oh,
            }
        )

    global _LAST_IN_MAPS
    _LAST_IN_MAPS = in_maps

    from concourse.bass_utils import run_bass_kernel_spmd

    res = run_bass_kernel_spmd(nc, in_maps, list(range(8)))
    out_T = res.results[0]["yout"]  # [D, N]
    return np.ascontiguousarray(out_T.T).reshape(B, S, D).astype(np.float32)



# revision 3
# speedup vs baseline: 1.4771x; 1.4771x over previous
"""MoE routing kernel for Trainium2 (8 NeuronCores, hybrid expert x token sharding).

Model (per layer l in 0..L-1):
    w = softmax(top-k masked |x @ protos[l].T|)          # [N, E] routing
    h = relu(x @ W1[l,e]); y = sum_e w[:,e] * (h @ W2[l,e])
    x = x + y

Sharding: 2 expert-groups x 4 token-shards.  Core c owns expert group
g = c // 4 (experts 4g..4g+3) and token shard t = c % 4 (512 tokens).
Each core computes the routing for its 512 tokens over all 8 experts
(prototypes replicated, exact f32r), then runs its 4 experts' FFNs in
bf16 (weights streamed from HBM, hidden under the matmuls), accumulating
w-scaled partial sums in fp32.  Cores with g == 0 fold in the residual x.
A pair-wise AllReduce (c <-> c+4) completes y for the shard, and its
output is the next layer's input.  No all-8 collective anywhere.

Numerics: routing scores in f32r (exact top-k vs the fp32 reference);
FFN matmuls in bf16 with fp32 PSUM accumulation (measured ~5e-3 rel err
vs fp32, gate is 2e-2).

The kernel is built once and cached at module level; repeated kernel()
calls reuse the compiled executable.
"""

import ml_dtypes
import numpy as np

import concourse.bacc as bacc
import concourse.mybir as mybir
from concourse import tile
from concourse.masks import make_identity

P = 128


def build_moe(
    nc,
    D=1024,
    F=2048,
    NTOK=512,   # tokens per core (shard)
    E=8,
    EG=4,       # experts per core (group size)
    L=2,
    K=2,
):
    """Emit the SPMD MoE program into Bass instance `nc`."""
    DS = D // P        # 8  D-slices
    FS = F // P        # 16 F-slices
    TT = NTOK // P     # 4  token-tiles
    FCH = 512          # W1 f-chunk (stream granularity)
    DCH = 512          # W2 d-chunk
    NFC = F // FCH     # 4
    NDC = D // DCH     # 2
    f32 = mybir.dt.float32
    f32r = mybir.dt.float32r
    bf16 = mybir.dt.bfloat16

    xT = nc.dram_tensor("xT", [D, NTOK], f32, kind="ExternalInput")
    prot = nc.dram_tensor("prot", [L, D, E], f32r, kind="ExternalInput")
    w1 = nc.dram_tensor("w1", [L, EG, D, F], bf16, kind="ExternalInput")
    w2 = nc.dram_tensor("w2", [L, EG, F, D], bf16, kind="ExternalInput")
    alpha = nc.dram_tensor("alpha", [P, 1], f32, kind="ExternalInput")
    esel = nc.dram_tensor("esel", [E, EG], f32, kind="ExternalInput")
    yout = nc.dram_tensor("yout", [D, NTOK], f32, kind="ExternalOutput")

    with tile.TileContext(nc) as tc:
        with (
            tc.tile_pool(name="const", bufs=1) as constp,
            tc.tile_pool(name="protp", bufs=1) as protp,
            tc.tile_pool(name="xpool", bufs=1) as xpool,
            tc.tile_pool(name="route", bufs=2) as routep,
            tc.tile_pool(name="wbcp", bufs=1) as wbcp,
            tc.tile_pool(name="w1p", bufs=3) as w1p,
            tc.tile_pool(name="w2p", bufs=3) as w2p,
            tc.tile_pool(name="hpool", bufs=2) as hpool,
            tc.tile_pool(name="ypool", bufs=1) as ypool,
            tc.tile_pool(name="tmpp", bufs=3) as tmpp,
            tc.tile_pool(name="psr", bufs=2, space="PSUM") as psr,
            tc.tile_pool(name="psh", bufs=2, space="PSUM") as psh,
            tc.tile_pool(name="psy", bufs=2, space="PSUM") as psy,
            tc.tile_pool(name="dram", bufs=2, space="DRAM") as dramp,
        ):
            ident = constp.tile([P, P], f32)
            make_identity(nc, ident)
            alpha_sb = constp.tile([P, 1], f32)
            nc.sync.dma_start(alpha_sb[:], alpha.ap()[:])
            esel_sb = constp.tile([E, EG], f32)
            nc.sync.dma_start(esel_sb[:], esel.ap()[:])
            # expert-row selector, broadcast along the stationary free dim:
            # esel_full[j, e, m] = 1 iff j == 4*g + e   (same for all m)
            esel_full = constp.tile([E, EG, P], f32)
            nc.vector.tensor_copy(
                esel_full[:],
                esel_sb[:].rearrange("p (e o) -> p e o", o=1).to_broadcast([E, EG, P]),
            )

            xsrc = xT.ap()
            for l in range(L):
                prot_sb = protp.tile([P, DS, E], f32r, tag="prot")
                nc.sync.dma_start(
                    prot_sb[:], prot.ap()[l].rearrange("(t p) e -> p t e", p=P)
                )

                xb = xpool.tile([P, DS, NTOK], f32, tag="xb")
                for ds in range(DS):
                    nc.sync.dma_start(
                        xb[:, ds, :], xsrc[ds * P : (ds + 1) * P, :]
                    )
                xr = xpool.tile([P, DS, NTOK], f32r, tag="xr")
                nc.vector.tensor_copy(xr[:], xb[:])
                xh = xpool.tile([P, DS, NTOK], bf16, tag="xh")
                nc.vector.tensor_copy(xh[:], xb[:])

                # ---- routing over all E experts for my NTOK tokens ----
                ps_s = psr.tile([E, NTOK], f32, tag="psr")
                for ds in range(DS):
                    nc.tensor.matmul(
                        ps_s[:],
                        prot_sb[:, ds, :],
                        xr[:, ds, :],
                        start=(ds == 0),
                        stop=(ds == DS - 1),
                    )
                s_abs = routep.tile([E, NTOK], f32, tag="sabs")
                nc.scalar.activation(
                    s_abs[:], ps_s[:], mybir.ActivationFunctionType.Abs
                )
                s_tok = routep.tile([P, TT, E], f32, tag="stok")
                for tt in range(TT):
                    ps_t = psr.tile([P, E], f32, tag="psr")
                    nc.tensor.transpose(
                        ps_t[:], s_abs[:, tt * P : (tt + 1) * P], ident[:E, :E]
                    )
                    nc.scalar.copy(s_tok[:, tt, :], ps_t[:])
                srt = routep.tile([P, TT, E], f32, tag="srt")
                for tt in range(TT):
                    nc.vector.max(srt[:, tt, :], s_tok[:, tt, :])
                shif = routep.tile([P, TT, E], f32, tag="shif")
                nc.vector.tensor_tensor(
                    out=shif[:],
                    in0=s_tok[:],
                    in1=srt[:, :, 0:1].to_broadcast([P, TT, E]),
                    op=mybir.AluOpType.subtract,
                )
                ex = routep.tile([P, TT, E], f32, tag="ex")
                nc.scalar.activation(
                    ex[:], shif[:], mybir.ActivationFunctionType.Exp
                )
                mask = routep.tile([P, TT, E], f32, tag="mask")
                nc.vector.tensor_tensor(
                    out=mask[:],
                    in0=s_tok[:],
                    in1=srt[:, :, K - 1 : K].to_broadcast([P, TT, E]),
                    op=mybir.AluOpType.is_ge,
                )
                nc.vector.tensor_tensor(
                    out=ex[:], in0=ex[:], in1=mask[:], op=mybir.AluOpType.mult
                )
                den = routep.tile([P, TT, 1], f32, tag="den")
                nc.vector.reduce_sum(den[:], ex[:], axis=mybir.AxisListType.X)
                rec = routep.tile([P, TT, 1], f32, tag="rec")
                nc.vector.reciprocal(rec[:], den[:])
                wtok = routep.tile([P, TT, E], f32, tag="wtok")
                nc.vector.tensor_tensor(
                    out=wtok[:],
                    in0=ex[:],
                    in1=rec[:].to_broadcast([P, TT, E]),
                    op=mybir.AluOpType.mult,
                )
                # expert-major weights: wrow[j, tok] for all 8 experts
                wrow = routep.tile([E, NTOK], f32, tag="wrow")
                for tt in range(TT):
                    ps_w = psr.tile([E, P], f32, tag="psr")
                    nc.tensor.transpose(ps_w[:], wtok[:, tt, :], ident[:])
                    nc.scalar.copy(wrow[:, tt * P : (tt + 1) * P], ps_w[:])
                # broadcast my EG experts' rows to all partitions
                wbc = wbcp.tile([P, EG, NTOK], f32, tag="wbc")
                for e in range(EG):
                    ps_b = psr.tile([P, NTOK], f32, tag="psr")
                    nc.tensor.matmul(
                        ps_b[:], esel_full[:, e, :], wrow[:], start=True, stop=True
                    )
                    nc.scalar.copy(wbc[:, e, :], ps_b[:])

                # ---- FFN: my EG experts over my NTOK tokens ----
                y_sb = ypool.tile([P, DS, NTOK], f32, tag="ysb")
                nc.scalar.activation(
                    y_sb[:], xb[:], mybir.ActivationFunctionType.Copy,
                    scale=alpha_sb[:, 0:1],
                )
                for e in range(EG):
                    h = hpool.tile([P, FS, NTOK], bf16, tag="h")
                    for fc in range(NFC):
                        w1c = w1p.tile([P, DS, FCH], bf16, tag="w1c")
                        for ds in range(DS):
                            nc.sync.dma_start(
                                w1c[:, ds, :],
                                w1.ap()[
                                    l, e,
                                    ds * P : (ds + 1) * P,
                                    fc * FCH : (fc + 1) * FCH,
                                ],
                            )
                        for fj in range(FCH // P):
                            fs = fc * (FCH // P) + fj
                            ps_h = psh.tile([P, NTOK], f32, tag="psh")
                            for ds in range(DS):
                                nc.tensor.matmul(
                                    ps_h[:],
                                    w1c[:, ds, fj * P : (fj + 1) * P],
                                    xh[:, ds, :],
                                    start=(ds == 0),
                                    stop=(ds == DS - 1),
                                )
                            nc.scalar.activation(
                                h[:, fs, :], ps_h[:],
                                mybir.ActivationFunctionType.Relu,
                            )
                    for dc in range(NDC):
                        w2c = w2p.tile([P, FS, DCH], bf16, tag="w2c")
                        for fs in range(FS):
                            nc.sync.dma_start(
                                w2c[:, fs, :],
                                w2.ap()[
                                    l, e,
                                    fs * P : (fs + 1) * P,
                                    dc * DCH : (dc + 1) * DCH,
                                ],
                            )
                        for dj in range(DCH // P):
                            ds = dc * (DCH // P) + dj
                            ps_y = psy.tile([P, NTOK], f32, tag="psy")
                            for fs in range(FS):
                                nc.tensor.matmul(
                                    ps_y[:],
                                    w2c[:, fs, dj * P : (dj + 1) * P],
                                    h[:, fs, :],
                                    start=(fs == 0),
                                    stop=(fs == FS - 1),
                                )
                            tmp = tmpp.tile([P, NTOK], f32, tag="tmp")
                            nc.vector.tensor_tensor(
                                out=tmp[:],
                                in0=ps_y[:],
                                in1=wbc[:, e, :],
                                op=mybir.AluOpType.mult,
                            )
                            nc.vector.tensor_tensor(
                                out=y_sb[:, ds, :],
                                in0=y_sb[:, ds, :],
                                in1=tmp[:],
                                op=mybir.AluOpType.add,
                            )

                # ---- pair AllReduce: (c, c+4) share the token shard ----
                ypart = dramp.tile(
                    [D, NTOK], f32, tag="ypart", name=f"ypart_{l}"
                )
                ysum = dramp.tile(
                    [D, NTOK], f32, tag="ysum", name=f"ysum_{l}"
                )
                for ds in range(DS):
                    nc.sync.dma_start(
                        ypart[ds * P : (ds + 1) * P, :], y_sb[:, ds, :]
                    )
                nc.gpsimd.collective_compute(
                    "AllReduce",
                    mybir.AluOpType.add,
                    replica_groups=[[0, 4], [1, 5], [2, 6], [3, 7]],
                    ins=[ypart[:]],
                    outs=[ysum[:]],
                )
                xsrc = ysum

            nc.sync.dma_start(yout.ap()[:], xsrc[:])
    return nc


_CACHE = {}


def _get_compiled():
    if "nc" not in _CACHE:
        nc = bacc.Bacc("TRN2", target_bir_lowering=False, debug=False, num_devices=8)
        build_moe(nc)
        nc.compile()
        _CACHE["nc"] = nc
    return _CACHE["nc"]


def kernel(x, protos, W1, W2, k):
    assert int(k) == 2
    B, S, Dx = x.shape
    L, E, D, F = W1.shape[0], W1.shape[1], W1.shape[2], W1.shape[3]
    N = B * S
    assert (B, S, Dx, L, E, D, F) == (2, 1024, 1024, 2, 8, 1024, 2048)
    NTOK = N // 4  # tokens per shard
    EG = E // 2    # experts per group

    nc = _get_compiled()

    xT = np.ascontiguousarray(np.asarray(x, dtype=np.float32).reshape(N, D).T)
    protT = np.ascontiguousarray(
        np.asarray(protos, dtype=np.float32).transpose(0, 2, 1)
    )
    W1b = np.asarray(W1, dtype=np.float32).astype(ml_dtypes.bfloat16)
    W2b = np.asarray(W2, dtype=np.float32).astype(ml_dtypes.bfloat16)

    in_maps = []
    for c in range(8):
        g, t = c // 4, c % 4
        alpha = np.full((P, 1), 1.0 if g == 0 else 0.0, dtype=np.float32)
        es = np.zeros((E, EG), dtype=np.float32)
        for e in range(EG):
            es[g * EG + e, e] = 1.0
        in_maps.append(
            {
                "xT": np.ascontiguousarray(xT[:, t * NTOK : (t + 1) * NTOK]),
                "prot": protT,
                "w1": np.ascontiguousarray(W1b[:, g * EG : (g + 1) * EG]),
                "w2": np.ascontiguousarray(W2b[:, g * EG : (g + 1) * EG]),
                "alpha": alpha,
                "esel": es,
            }
        )

    global _LAST_IN_MAPS
    _LAST_IN_MAPS = in_maps

    from concourse.bass_utils import run_bass_kernel_spmd

    res = run_bass_kernel_spmd(nc, in_maps, list(range(8)))
    out_T = np.empty((D, N), dtype=np.float32)
    for t in range(4):
        out_T[:, t * NTOK : (t + 1) * NTOK] = res.results[t]["yout"]
    return np.ascontiguousarray(out_T.T).reshape(B, S, D).astype(np.float32)


# revision 6
# speedup vs baseline: 1.5279x; 1.0344x over previous
"""MoE routing kernel for Trainium2 (8 NeuronCores, hybrid expert x token sharding).

Model (per layer l in 0..L-1):
    w = softmax(top-k masked |x @ protos[l].T|)          # [N, E] routing
    h = relu(x @ W1[l,e]); y = sum_e w[:,e] * (h @ W2[l,e])
    x = x + y

Sharding: 2 expert-groups x 4 token-shards.  Core c owns expert group
g = c // 4 (experts 4g..4g+3) and token shard t = c % 4 (512 tokens).
Each core computes the routing for its 512 tokens over all 8 experts
(prototypes replicated, exact f32r), runs its 4 experts' FFNs in bf16
(weights streamed from HBM, hidden under the matmuls), accumulating
w-scaled partial sums in fp32.  Cores with g == 0 fold in the residual.
A pair-wise AllReduce (c <-> c+4) completes y for the shard.

Bubble-avoidance:
  * collectives split per D-half so the first half's exchange overlaps the
    last expert's second-half compute;
  * layer-2 routing runs on PARTIAL sums: each core computes scores from its
    y partial, a 16KB score-AllReduce completes them, and the whole softmax /
    top-k chain hides under the big activation AllReduce;
  * the final layer uses ReduceScatter straight into the output tensor
    (halving the tail exchange); the host reassembles the D-halves.

Numerics: routing scores in f32r (exact top-k vs the fp32 reference);
FFN matmuls in bf16 with fp32 PSUM accumulation (measured ~5e-3 rel err
vs fp32, gate is 2e-2).
"""

import ml_dtypes
import numpy as np

import concourse.bacc as bacc
import concourse.mybir as mybir
from concourse import tile
from concourse.masks import make_identity

P = 128


def build_moe(
    nc,
    D=1024,
    F=2048,
    NTOK=512,   # tokens per core (shard)
    E=8,
    EG=4,       # experts per core (group size)
    L=2,
    K=2,
):
    """Emit the SPMD MoE program into Bass instance `nc`."""
    DS = D // P        # 8  D-slices
    FS = F // P        # 16 F-slices
    TT = NTOK // P     # 4  token-tiles
    FCH = 512          # W1 f-chunk (stream granularity)
    DCH = 512          # W2 d-chunk
    NFC = F // FCH     # 4
    NDC = D // DCH     # 2
    DJ = DCH // P      # 4  d-tiles per chunk
    f32 = mybir.dt.float32
    f32r = mybir.dt.float32r
    bf16 = mybir.dt.bfloat16
    PAIRS = [[0, 4], [1, 5], [2, 6], [3, 7]]

    xT = nc.dram_tensor("xT", [D, NTOK], f32, kind="ExternalInput")
    prot = nc.dram_tensor("prot", [L, D, E], f32r, kind="ExternalInput")
    w1 = nc.dram_tensor("w1", [L, EG, D, F], bf16, kind="ExternalInput")
    w2 = nc.dram_tensor("w2", [L, EG, F, D], bf16, kind="ExternalInput")
    alpha = nc.dram_tensor("alpha", [P, 1], f32, kind="ExternalInput")
    esel = nc.dram_tensor("esel", [E, EG], f32, kind="ExternalInput")
    # final output: this core's ReduceScatter halves, [dc, 256, NTOK] packed
    # as rows [0:256]=d-quarter from dc0, [256:512]=d-quarter from dc1.
    yout = nc.dram_tensor("yout", [DCH, NTOK], f32, kind="ExternalOutput")

    with tile.TileContext(nc) as tc:
        with (
            tc.tile_pool(name="const", bufs=1) as constp,
            tc.tile_pool(name="xpool", bufs=1) as xpool,
            tc.tile_pool(name="route", bufs=2) as routep,
            tc.tile_pool(name="wbcp", bufs=2) as wbcp,
            tc.tile_pool(name="w1p", bufs=3) as w1p,
            tc.tile_pool(name="w2p", bufs=2) as w2p,
            tc.tile_pool(name="hpool", bufs=2) as hpool,
            tc.tile_pool(name="ypool", bufs=1) as ypool,
            tc.tile_pool(name="tmpp", bufs=3) as tmpp,
            tc.tile_pool(name="psr", bufs=2, space="PSUM") as psr,
            tc.tile_pool(name="psh", bufs=2, space="PSUM") as psh,
            tc.tile_pool(name="psy", bufs=2, space="PSUM") as psy,
            tc.tile_pool(name="dram", bufs=1, space="DRAM") as dramp,
        ):
            ident = constp.tile([P, P], f32)
            make_identity(nc, ident)
            alpha_sb = constp.tile([P, 1], f32)
            nc.sync.dma_start(alpha_sb[:], alpha.ap()[:])
            esel_sb = constp.tile([E, EG], f32)
            nc.sync.dma_start(esel_sb[:], esel.ap()[:])
            # expert-row selector, broadcast along the stationary free dim:
            # esel_full[j, e, m] = 1 iff j == 4*g + e   (same for all m)
            esel_full = constp.tile([E, EG, P], f32)
            nc.vector.tensor_copy(
                esel_full[:],
                esel_sb[:].rearrange("p (e o) -> p e o", o=1).to_broadcast([E, EG, P]),
            )
            prots = []
            for l in range(L):
                psb = constp.tile([P, DS, E], f32r, name=f"prot{l}")
                nc.sync.dma_start(
                    psb[:], prot.ap()[l].rearrange("(t p) e -> p t e", p=P)
                )
                prots.append(psb)

            def routing_chain(s_abs, lbl):
                """softmax(top-K masked scores) -> wbc [P, EG, NTOK]."""
                s_tok = routep.tile([P, TT, E], f32, tag="stok")
                for tt in range(TT):
                    ps_t = psr.tile([P, E], f32, tag="psr")
                    nc.tensor.transpose(
                        ps_t[:], s_abs[:, tt * P : (tt + 1) * P], ident[:E, :E]
                    )
                    nc.scalar.copy(s_tok[:, tt, :], ps_t[:])
                srt = routep.tile([P, TT, E], f32, tag="srt")
                for tt in range(TT):
                    nc.vector.max(srt[:, tt, :], s_tok[:, tt, :])
                shif = routep.tile([P, TT, E], f32, tag="shif")
                nc.vector.tensor_tensor(
                    out=shif[:],
                    in0=s_tok[:],
                    in1=srt[:, :, 0:1].to_broadcast([P, TT, E]),
                    op=mybir.AluOpType.subtract,
                )
                ex = routep.tile([P, TT, E], f32, tag="ex")
                nc.scalar.activation(
                    ex[:], shif[:], mybir.ActivationFunctionType.Exp
                )
                mask = routep.tile([P, TT, E], f32, tag="mask")
                nc.vector.tensor_tensor(
                    out=mask[:],
                    in0=s_tok[:],
                    in1=srt[:, :, K - 1 : K].to_broadcast([P, TT, E]),
                    op=mybir.AluOpType.is_ge,
                )
                nc.vector.tensor_tensor(
                    out=ex[:], in0=ex[:], in1=mask[:], op=mybir.AluOpType.mult
                )
                den = routep.tile([P, TT, 1], f32, tag="den")
                nc.vector.reduce_sum(den[:], ex[:], axis=mybir.AxisListType.X)
                rec = routep.tile([P, TT, 1], f32, tag="rec")
                nc.vector.reciprocal(rec[:], den[:])
                wtok = routep.tile([P, TT, E], f32, tag="wtok")
                nc.vector.tensor_tensor(
                    out=wtok[:],
                    in0=ex[:],
                    in1=rec[:].to_broadcast([P, TT, E]),
                    op=mybir.AluOpType.mult,
                )
                wrow = routep.tile([E, NTOK], f32, tag="wrow")
                for tt in range(TT):
                    ps_w = psr.tile([E, P], f32, tag="psr")
                    nc.tensor.transpose(ps_w[:], wtok[:, tt, :], ident[:])
                    nc.scalar.copy(wrow[:, tt * P : (tt + 1) * P], ps_w[:])
                wbc = wbcp.tile([P, EG, NTOK], f32, tag="wbc", name=f"wbc_{lbl}")
                for e in range(EG):
                    ps_b = psr.tile([P, NTOK], f32, tag="psr")
                    nc.tensor.matmul(
                        ps_b[:], esel_full[:, e, :], wrow[:], start=True, stop=True
                    )
                    nc.scalar.copy(wbc[:, e, :], ps_b[:])
                return wbc

            def ffn(l, xh, wbc, ysb):
                """4 experts' FFN over this shard; accumulate into ysb halves."""
                for e in range(EG):
                    h = hpool.tile([P, FS, NTOK], bf16, tag="h")
                    for fc in range(NFC):
                        w1c = w1p.tile([P, DS, FCH], bf16, tag="w1c")
                        for ds in range(DS):
                            nc.sync.dma_start(
                                w1c[:, ds, :],
                                w1.ap()[
                                    l, e,
                                    ds * P : (ds + 1) * P,
                                    fc * FCH : (fc + 1) * FCH,
                                ],
                            )
                        for fj in range(FCH // P):
                            fs = fc * (FCH // P) + fj
                            ps_h = psh.tile([P, NTOK], f32, tag="psh")
                            for ds in range(DS):
                                nc.tensor.matmul(
                                    ps_h[:],
                                    w1c[:, ds, fj * P : (fj + 1) * P],
                                    xh[:, ds, :],
                                    start=(ds == 0),
                                    stop=(ds == DS - 1),
                                )
                            nc.scalar.activation(
                                h[:, fs, :], ps_h[:],
                                mybir.ActivationFunctionType.Relu,
                            )
                    for dc in range(NDC):
                        w2c = w2p.tile([P, FS, DCH], bf16, tag="w2c")
                        for fs in range(FS):
                            nc.sync.dma_start(
                                w2c[:, fs, :],
                                w2.ap()[
                                    l, e,
                                    fs * P : (fs + 1) * P,
                                    dc * DCH : (dc + 1) * DCH,
                                ],
                            )
                        for dj in range(DJ):
                            ps_y = psy.tile([P, NTOK], f32, tag="psy")
                            for fs in range(FS):
                                nc.tensor.matmul(
                                    ps_y[:],
                                    w2c[:, fs, dj * P : (dj + 1) * P],
                                    h[:, fs, :],
                                    start=(fs == 0),
                                    stop=(fs == FS - 1),
                                )
                            tmp = tmpp.tile([P, NTOK], f32, tag="tmp")
                            nc.vector.tensor_tensor(
                                out=tmp[:],
                                in0=ps_y[:],
                                in1=wbc[:, e, :],
                                op=mybir.AluOpType.mult,
                            )
                            nc.vector.tensor_tensor(
                                out=ysb[dc][:, dj, :],
                                in0=ysb[dc][:, dj, :],
                                in1=tmp[:],
                                op=mybir.AluOpType.add,
                            )

            # ================= layer 0 =================
            xb = xpool.tile([P, DS, NTOK], f32, tag="xb")
            for ds in range(DS):
                nc.sync.dma_start(xb[:, ds, :], xT.ap()[ds * P : (ds + 1) * P, :])
            xr = xpool.tile([P, DS, NTOK], f32r, tag="xr")
            nc.vector.tensor_copy(xr[:], xb[:])
            xh0 = xpool.tile([P, DS, NTOK], bf16, tag="xh")
            nc.vector.tensor_copy(xh0[:], xb[:])

            ps_s = psr.tile([E, NTOK], f32, tag="psr")
            for ds in range(DS):
                nc.tensor.matmul(
                    ps_s[:], prots[0][:, ds, :], xr[:, ds, :],
                    start=(ds == 0), stop=(ds == DS - 1),
                )
            s_abs0 = routep.tile([E, NTOK], f32, tag="sabs")
            nc.scalar.activation(
                s_abs0[:], ps_s[:], mybir.ActivationFunctionType.Abs
            )
            wbc0 = routing_chain(s_abs0, "l0")

            ysb0 = []
            for dc in range(NDC):
                y = ypool.tile([P, DJ, NTOK], f32, tag=f"ysb{dc}", name=f"ysb0_{dc}")
                nc.scalar.activation(
                    y[:], xb[:, dc * DJ : (dc + 1) * DJ, :],
                    mybir.ActivationFunctionType.Copy, scale=alpha_sb[:, 0:1],
                )
                ysb0.append(y)

            ffn(0, xh0, wbc0, ysb0)

            # ---- transition: per-half AllReduce + pre-AR routing for layer 1
            yp0 = [
                dramp.tile([DCH, NTOK], f32, tag=f"yp0_{dc}", name=f"yp0_{dc}")
                for dc in range(NDC)
            ]
            ys0 = [
                dramp.tile([DCH, NTOK], f32, tag=f"ys0_{dc}", name=f"ys0_{dc}")
                for dc in range(NDC)
            ]
            for dc in range(NDC):
                for dj in range(DJ):
                    nc.sync.dma_start(
                        yp0[dc][dj * P : (dj + 1) * P, :], ysb0[dc][:, dj, :]
                    )
            # partial-sum scores for layer 1 (completed by a tiny AllReduce)
            ysr = xpool.tile([P, DS, NTOK], f32r, tag="xr")
            for dc in range(NDC):
                nc.vector.tensor_copy(
                    ysr[:, dc * DJ : (dc + 1) * DJ, :], ysb0[dc][:]
                )
            ps_sp = psr.tile([E, NTOK], f32, tag="psr")
            for ds in range(DS):
                nc.tensor.matmul(
                    ps_sp[:], prots[1][:, ds, :], ysr[:, ds, :],
                    start=(ds == 0), stop=(ds == DS - 1),
                )
            spre = routep.tile([E, NTOK], f32, tag="spre")
            nc.scalar.copy(spre[:], ps_sp[:])
            sp_d = dramp.tile([E, NTOK], f32, tag="sp_d", name="sp_d")
            ss_d = dramp.tile([E, NTOK], f32, tag="ss_d", name="ss_d")
            nc.sync.dma_start(sp_d[:], spre[:])

            # CC issue order: big half 0 first (its data is ready earliest),
            # then the tiny score AR, then big half 1.
            nc.gpsimd.collective_compute(
                "AllReduce", mybir.AluOpType.add, replica_groups=PAIRS,
                ins=[yp0[0][:]], outs=[ys0[0][:]],
            )
            nc.gpsimd.collective_compute(
                "AllReduce", mybir.AluOpType.add, replica_groups=PAIRS,
                ins=[sp_d[:]], outs=[ss_d[:]],
            )
            nc.gpsimd.collective_compute(
                "AllReduce", mybir.AluOpType.add, replica_groups=PAIRS,
                ins=[yp0[1][:]], outs=[ys0[1][:]],
            )
            ssum = routep.tile([E, NTOK], f32, tag="ssum")
            nc.sync.dma_start(ssum[:], ss_d[:])
            s_abs1 = routep.tile([E, NTOK], f32, tag="sabs")
            nc.scalar.activation(
                s_abs1[:], ssum[:], mybir.ActivationFunctionType.Abs
            )
            wbc1 = routing_chain(s_abs1, "l1")

            # ================= layer 1 =================
            xb1 = xpool.tile([P, DS, NTOK], f32, tag="xb")
            for dc in range(NDC):
                for dj in range(DJ):
                    nc.sync.dma_start(
                        xb1[:, dc * DJ + dj, :], ys0[dc][dj * P : (dj + 1) * P, :]
                    )
            xh1 = xpool.tile([P, DS, NTOK], bf16, tag="xh")
            for dc in range(NDC):
                nc.vector.tensor_copy(
                    xh1[:, dc * DJ : (dc + 1) * DJ, :],
                    xb1[:, dc * DJ : (dc + 1) * DJ, :],
                )
            ysb1 = []
            for dc in range(NDC):
                y = ypool.tile([P, DJ, NTOK], f32, tag=f"ysb{dc}", name=f"ysb1_{dc}")
                nc.scalar.activation(
                    y[:], xb1[:, dc * DJ : (dc + 1) * DJ, :],
                    mybir.ActivationFunctionType.Copy, scale=alpha_sb[:, 0:1],
                )
                ysb1.append(y)

            ffn(1, xh1, wbc1, ysb1)

            # ---- tail: per-half ReduceScatter straight into yout
            yp1 = [
                dramp.tile([DCH, NTOK], f32, tag=f"yp1_{dc}", name=f"yp1_{dc}")
                for dc in range(NDC)
            ]
            for dc in range(NDC):
                for dj in range(DJ):
                    nc.sync.dma_start(
                        yp1[dc][dj * P : (dj + 1) * P, :], ysb1[dc][:, dj, :]
                    )
            HQ = DCH // 2  # 256 rows received per core per half
            yf = [
                dramp.tile([HQ, NTOK], f32, tag=f"yf_{dc}", name=f"yf_{dc}")
                for dc in range(NDC)
            ]
            for dc in range(NDC):
                nc.gpsimd.collective_compute(
                    "ReduceScatter", mybir.AluOpType.add, replica_groups=PAIRS,
                    ins=[yp1[dc][:]], outs=[yf[dc][:]],
                )
            for dc in range(NDC):
                nc.sync.dma_start(yout.ap()[dc * HQ : (dc + 1) * HQ, :], yf[dc][:])
    return nc


_CACHE = {}


def _get_compiled():
    if "nc" not in _CACHE:
        nc = bacc.Bacc("TRN2", target_bir_lowering=False, debug=False, num_devices=8)
        build_moe(nc)
        nc.compile()
        _CACHE["nc"] = nc
    return _CACHE["nc"]


def kernel(x, protos, W1, W2, k):
    assert int(k) == 2
    B, S, Dx = x.shape
    L, E, D, F = W1.shape[0], W1.shape[1], W1.shape[2], W1.shape[3]
    N = B * S
    assert (B, S, Dx, L, E, D, F) == (2, 1024, 1024, 2, 8, 1024, 2048)
    NTOK = N // 4  # tokens per shard
    EG = E // 2    # experts per group

    nc = _get_compiled()

    xT = np.ascontiguousarray(np.asarray(x, dtype=np.float32).reshape(N, D).T)
    protT = np.ascontiguousarray(
        np.asarray(protos, dtype=np.float32).transpose(0, 2, 1)
    )
    W1b = np.asarray(W1, dtype=np.float32).astype(ml_dtypes.bfloat16)
    W2b = np.asarray(W2, dtype=np.float32).astype(ml_dtypes.bfloat16)

    in_maps = []
    for c in range(8):
        g, t = c // 4, c % 4
        alpha = np.full((P, 1), 1.0 if g == 0 else 0.0, dtype=np.float32)
        es = np.zeros((E, EG), dtype=np.float32)
        for e in range(EG):
            es[g * EG + e, e] = 1.0
        in_maps.append(
            {
                "xT": np.ascontiguousarray(xT[:, t * NTOK : (t + 1) * NTOK]),
                "prot": protT,
                "w1": np.ascontiguousarray(W1b[:, g * EG : (g + 1) * EG]),
                "w2": np.ascontiguousarray(W2b[:, g * EG : (g + 1) * EG]),
                "alpha": alpha,
                "esel": es,
            }
        )

    global _LAST_IN_MAPS
    _LAST_IN_MAPS = in_maps

    from concourse.bass_utils import run_bass_kernel_spmd

    res = run_bass_kernel_spmd(nc, in_maps, list(range(8)))
    # yout layout per core: rows [0:256] = its quarter of d-half 0,
    # rows [256:512] = its quarter of d-half 1.  Rank 0 of each pair (g=0)
    # receives the first quarter of each half.
    HQ = 256
    out_T = np.empty((D, N), dtype=np.float32)
    for t in range(4):
        lo = res.results[t]["yout"]        # g=0 core: d [0:256] and [512:768]
        hi = res.results[t + 4]["yout"]    # g=1 core: d [256:512] and [768:1024]
        sl = slice(t * NTOK, (t + 1) * NTOK)
        out_T[0:256, sl] = lo[0:HQ]
        out_T[256:512, sl] = hi[0:HQ]
        out_T[512:768, sl] = lo[HQ:]
        out_T[768:1024, sl] = hi[HQ:]
    return np.ascontiguousarray(out_T.T).reshape(B, S, D).astype(np.float32)


# revision 8
# speedup vs baseline: 1.5876x; 1.0391x over previous
"""MoE routing kernel for Trainium2 (8 NeuronCores, hybrid expert x token sharding).

Model (per layer l in 0..L-1):
    w = softmax(top-k masked |x @ protos[l].T|)          # [N, E] routing
    h = relu(x @ W1[l,e]); y = sum_e w[:,e] * (h @ W2[l,e])
    x = x + y

Sharding: 2 expert-groups x 4 token-shards.  Core c owns expert group
g = c // 4 (experts 4g..4g+3) and token shard t = c % 4 (512 tokens).
Each core computes the routing for its 512 tokens over all 8 experts
(prototypes replicated, exact f32r), runs its 4 experts' FFNs in bf16
(weights streamed from HBM, hidden under the matmuls), accumulating
w-scaled partial sums in fp32.  Cores with g == 0 fold in the residual.
A pair-wise AllReduce (c <-> c+4) completes y for the shard.

Bubble-avoidance:
  * collectives split per D-half so the first half's exchange overlaps the
    last expert's second-half compute;
  * layer-2 routing runs on PARTIAL sums: each core computes scores from its
    y partial, a 16KB score-AllReduce completes them, and the whole softmax /
    top-k chain hides under the big activation AllReduce;
  * the final layer uses ReduceScatter straight into the output tensor
    (halving the tail exchange); the host reassembles the D-halves.

Numerics: routing scores in f32r (exact top-k vs the fp32 reference);
FFN matmuls in bf16 with fp32 PSUM accumulation (measured ~5e-3 rel err
vs fp32, gate is 2e-2).
"""

import ml_dtypes
import numpy as np

import concourse.bacc as bacc
import concourse.mybir as mybir
from concourse import tile
from concourse.masks import make_identity

P = 128


def build_moe(
    nc,
    D=1024,
    F=2048,
    NTOK=512,   # tokens per core (shard)
    E=8,
    EG=4,       # experts per core (group size)
    L=2,
    K=2,
):
    """Emit the SPMD MoE program into Bass instance `nc`."""
    DS = D // P        # 8  D-slices
    FS = F // P        # 16 F-slices
    TT = NTOK // P     # 4  token-tiles
    FCH = 512          # W1 f-chunk (stream granularity)
    DCH = 512          # W2 d-chunk
    NFC = F // FCH     # 4
    NDC = D // DCH     # 2
    DJ = DCH // P      # 4  d-tiles per chunk
    f32 = mybir.dt.float32
    f32r = mybir.dt.float32r
    bf16 = mybir.dt.bfloat16
    PAIRS = [[0, 4], [1, 5], [2, 6], [3, 7]]

    xT = nc.dram_tensor("xT", [D, NTOK], f32, kind="ExternalInput")
    xTh = nc.dram_tensor("xTh", [D, NTOK], bf16, kind="ExternalInput")
    prot = nc.dram_tensor("prot", [L, D, E], f32r, kind="ExternalInput")
    w1 = nc.dram_tensor("w1", [L, EG, D, F], bf16, kind="ExternalInput")
    w2 = nc.dram_tensor("w2", [L, EG, F, D], bf16, kind="ExternalInput")
    alpha = nc.dram_tensor("alpha", [P, 1], f32, kind="ExternalInput")
    esel = nc.dram_tensor("esel", [E, EG], f32, kind="ExternalInput")
    # final output: this core's ReduceScatter halves, [dc, 256, NTOK] packed
    # as rows [0:256]=d-quarter from dc0, [256:512]=d-quarter from dc1.
    yout = nc.dram_tensor("yout", [DCH, NTOK], bf16, kind="ExternalOutput")

    with tile.TileContext(nc) as tc:
        with (
            tc.tile_pool(name="const", bufs=1) as constp,
            tc.tile_pool(name="xpool", bufs=1) as xpool,
            tc.tile_pool(name="route", bufs=2) as routep,
            tc.tile_pool(name="wbcp", bufs=2) as wbcp,
            tc.tile_pool(name="w1p", bufs=3) as w1p,
            tc.tile_pool(name="w2p", bufs=2) as w2p,
            tc.tile_pool(name="hpool", bufs=2) as hpool,
            tc.tile_pool(name="ypool", bufs=1) as ypool,
            tc.tile_pool(name="tmpp", bufs=3) as tmpp,
            tc.tile_pool(name="ybfp", bufs=1) as ybfp,
            tc.tile_pool(name="psr", bufs=2, space="PSUM") as psr,
            tc.tile_pool(name="psh", bufs=2, space="PSUM") as psh,
            tc.tile_pool(name="psy", bufs=2, space="PSUM") as psy,
            tc.tile_pool(name="dram", bufs=1, space="DRAM") as dramp,
        ):
            ident = constp.tile([P, P], f32)
            make_identity(nc, ident)
            alpha_sb = constp.tile([P, 1], f32)
            nc.sync.dma_start(alpha_sb[:], alpha.ap()[:])
            esel_sb = constp.tile([E, EG], f32)
            nc.sync.dma_start(esel_sb[:], esel.ap()[:])
            # expert-row selector, broadcast along the stationary free dim:
            # esel_full[j, e, m] = 1 iff j == 4*g + e   (same for all m)
            esel_full = constp.tile([E, EG, P], f32)
            nc.vector.tensor_copy(
                esel_full[:],
                esel_sb[:].rearrange("p (e o) -> p e o", o=1).to_broadcast([E, EG, P]),
            )
            prots = []
            for l in range(L):
                psb = constp.tile([P, DS, E], f32r, name=f"prot{l}")
                nc.sync.dma_start(
                    psb[:], prot.ap()[l].rearrange("(t p) e -> p t e", p=P)
                )
                prots.append(psb)

            def routing_chain(s_abs, lbl):
                """softmax(top-K masked scores) -> wbc [P, EG, NTOK]."""
                s_tok = routep.tile([P, TT, E], f32, tag="stok")
                for tt in range(TT):
                    ps_t = psr.tile([P, E], f32, tag="psr")
                    nc.tensor.transpose(
                        ps_t[:], s_abs[:, tt * P : (tt + 1) * P], ident[:E, :E]
                    )
                    nc.scalar.copy(s_tok[:, tt, :], ps_t[:])
                srt = routep.tile([P, TT, E], f32, tag="srt")
                for tt in range(TT):
                    nc.vector.max(srt[:, tt, :], s_tok[:, tt, :])
                shif = routep.tile([P, TT, E], f32, tag="shif")
                nc.vector.tensor_tensor(
                    out=shif[:],
                    in0=s_tok[:],
                    in1=srt[:, :, 0:1].to_broadcast([P, TT, E]),
                    op=mybir.AluOpType.subtract,
                )
                ex = routep.tile([P, TT, E], f32, tag="ex")
                nc.scalar.activation(
                    ex[:], shif[:], mybir.ActivationFunctionType.Exp
                )
                mask = routep.tile([P, TT, E], f32, tag="mask")
                nc.vector.tensor_tensor(
                    out=mask[:],
                    in0=s_tok[:],
                    in1=srt[:, :, K - 1 : K].to_broadcast([P, TT, E]),
                    op=mybir.AluOpType.is_ge,
                )
                nc.vector.tensor_tensor(
                    out=ex[:], in0=ex[:], in1=mask[:], op=mybir.AluOpType.mult
                )
                den = routep.tile([P, TT, 1], f32, tag="den")
                nc.vector.reduce_sum(den[:], ex[:], axis=mybir.AxisListType.X)
                rec = routep.tile([P, TT, 1], f32, tag="rec")
                nc.vector.reciprocal(rec[:], den[:])
                wtok = routep.tile([P, TT, E], f32, tag="wtok")
                nc.vector.tensor_tensor(
                    out=wtok[:],
                    in0=ex[:],
                    in1=rec[:].to_broadcast([P, TT, E]),
                    op=mybir.AluOpType.mult,
                )
                wrow = routep.tile([E, NTOK], f32, tag="wrow")
                for tt in range(TT):
                    ps_w = psr.tile([E, P], f32, tag="psr")
                    nc.tensor.transpose(ps_w[:], wtok[:, tt, :], ident[:])
                    nc.scalar.copy(wrow[:, tt * P : (tt + 1) * P], ps_w[:])
                wbc = wbcp.tile([P, EG, NTOK], f32, tag="wbc", name=f"wbc_{lbl}")
                for e in range(EG):
                    ps_b = psr.tile([P, NTOK], f32, tag="psr")
                    nc.tensor.matmul(
                        ps_b[:], esel_full[:, e, :], wrow[:], start=True, stop=True
                    )
                    nc.scalar.copy(wbc[:, e, :], ps_b[:])
                return wbc

            def ffn(l, xh, wbc, ysb):
                """4 experts' FFN over this shard; accumulate into ysb halves."""
                for e in range(EG):
                    h = hpool.tile([P, FS, NTOK], bf16, tag="h")
                    for fc in range(NFC):
                        w1c = w1p.tile([P, DS, FCH], bf16, tag="w1c")
                        for ds in range(DS):
                            nc.sync.dma_start(
                                w1c[:, ds, :],
                                w1.ap()[
                                    l, e,
                                    ds * P : (ds + 1) * P,
                                    fc * FCH : (fc + 1) * FCH,
                                ],
                            )
                        for fj in range(FCH // P):
                            fs = fc * (FCH // P) + fj
                            ps_h = psh.tile([P, NTOK], f32, tag="psh")
                            for ds in range(DS):
                                nc.tensor.matmul(
                                    ps_h[:],
                                    w1c[:, ds, fj * P : (fj + 1) * P],
                                    xh[:, ds, :],
                                    start=(ds == 0),
                                    stop=(ds == DS - 1),
                                )
                            nc.scalar.activation(
                                h[:, fs, :], ps_h[:],
                                mybir.ActivationFunctionType.Relu,
                            )
                    for dc in range(NDC):
                        w2c = w2p.tile([P, FS, DCH], bf16, tag="w2c")
                        for fs in range(FS):
                            nc.sync.dma_start(
                                w2c[:, fs, :],
                                w2.ap()[
                                    l, e,
                                    fs * P : (fs + 1) * P,
                                    dc * DCH : (dc + 1) * DCH,
                                ],
                            )
                        for dj in range(DJ):
                            ps_y = psy.tile([P, NTOK], f32, tag="psy")
                            for fs in range(FS):
                                nc.tensor.matmul(
                                    ps_y[:],
                                    w2c[:, fs, dj * P : (dj + 1) * P],
                                    h[:, fs, :],
                                    start=(fs == 0),
                                    stop=(fs == FS - 1),
                                )
                            tmp = tmpp.tile([P, NTOK], f32, tag="tmp")
                            nc.vector.tensor_tensor(
                                out=tmp[:],
                                in0=ps_y[:],
                                in1=wbc[:, e, :],
                                op=mybir.AluOpType.mult,
                            )
                            nc.vector.tensor_tensor(
                                out=ysb[dc][:, dj, :],
                                in0=ysb[dc][:, dj, :],
                                in1=tmp[:],
                                op=mybir.AluOpType.add,
                            )

            # ================= layer 0 =================
            xb = xpool.tile([P, DS, NTOK], f32, tag="xb")
            for ds in range(DS):
                nc.sync.dma_start(xb[:, ds, :], xT.ap()[ds * P : (ds + 1) * P, :])
            xr = xpool.tile([P, DS, NTOK], f32r, tag="xr")
            nc.vector.tensor_copy(xr[:], xb[:])
            xh0 = xpool.tile([P, DS, NTOK], bf16, tag="xh")
            for ds in range(DS):
                nc.sync.dma_start(xh0[:, ds, :], xTh.ap()[ds * P : (ds + 1) * P, :])

            ps_s = psr.tile([E, NTOK], f32, tag="psr")
            for ds in range(DS):
                nc.tensor.matmul(
                    ps_s[:], prots[0][:, ds, :], xr[:, ds, :],
                    start=(ds == 0), stop=(ds == DS - 1),
                )
            s_abs0 = routep.tile([E, NTOK], f32, tag="sabs")
            nc.scalar.activation(
                s_abs0[:], ps_s[:], mybir.ActivationFunctionType.Abs
            )
            wbc0 = routing_chain(s_abs0, "l0")

            ysb0 = []
            for dc in range(NDC):
                y = ypool.tile([P, DJ, NTOK], f32, tag=f"ysb{dc}", name=f"ysb0_{dc}")
                nc.scalar.activation(
                    y[:], xb[:, dc * DJ : (dc + 1) * DJ, :],
                    mybir.ActivationFunctionType.Copy, scale=alpha_sb[:, 0:1],
                )
                ysb0.append(y)

            ffn(0, xh0, wbc0, ysb0)

            # ---- transition: per-half AllReduce + pre-AR routing for layer 1
            yp0 = [
                dramp.tile([DCH, NTOK], bf16, tag=f"yp0_{dc}", name=f"yp0_{dc}")
                for dc in range(NDC)
            ]
            ys0 = [
                dramp.tile([DCH, NTOK], bf16, tag=f"ys0_{dc}", name=f"ys0_{dc}")
                for dc in range(NDC)
            ]
            for dc in range(NDC):
                ybf = ybfp.tile([P, DJ, NTOK], bf16, tag=f"ybf{dc}")
                nc.vector.tensor_copy(ybf[:], ysb0[dc][:])
                for dj in range(DJ):
                    nc.sync.dma_start(
                        yp0[dc][dj * P : (dj + 1) * P, :], ybf[:, dj, :]
                    )
            # partial-sum scores for layer 1 (completed by a tiny AllReduce)
            ysr = xpool.tile([P, DS, NTOK], f32r, tag="xr")
            for dc in range(NDC):
                nc.vector.tensor_copy(
                    ysr[:, dc * DJ : (dc + 1) * DJ, :], ysb0[dc][:]
                )
            ps_sp = psr.tile([E, NTOK], f32, tag="psr")
            for ds in range(DS):
                nc.tensor.matmul(
                    ps_sp[:], prots[1][:, ds, :], ysr[:, ds, :],
                    start=(ds == 0), stop=(ds == DS - 1),
                )
            spre = routep.tile([E, NTOK], f32, tag="spre")
            nc.scalar.copy(spre[:], ps_sp[:])
            sp_d = dramp.tile([E, NTOK], f32, tag="sp_d", name="sp_d")
            ss_d = dramp.tile([E, NTOK], f32, tag="ss_d", name="ss_d")
            nc.sync.dma_start(sp_d[:], spre[:])

            # CC issue order: big half 0 first (its data is ready earliest),
            # then the tiny score AR, then big half 1.
            nc.gpsimd.collective_compute(
                "AllReduce", mybir.AluOpType.add, replica_groups=PAIRS,
                ins=[yp0[0][:]], outs=[ys0[0][:]],
            )
            nc.gpsimd.collective_compute(
                "AllReduce", mybir.AluOpType.add, replica_groups=PAIRS,
                ins=[sp_d[:]], outs=[ss_d[:]],
            )
            nc.gpsimd.collective_compute(
                "AllReduce", mybir.AluOpType.add, replica_groups=PAIRS,
                ins=[yp0[1][:]], outs=[ys0[1][:]],
            )
            ssum = routep.tile([E, NTOK], f32, tag="ssum")
            nc.sync.dma_start(ssum[:], ss_d[:])
            s_abs1 = routep.tile([E, NTOK], f32, tag="sabs")
            nc.scalar.activation(
                s_abs1[:], ssum[:], mybir.ActivationFunctionType.Abs
            )
            wbc1 = routing_chain(s_abs1, "l1")

            # ================= layer 1 =================
            xh1 = xpool.tile([P, DS, NTOK], bf16, tag="xh")
            for dc in range(NDC):
                for dj in range(DJ):
                    nc.sync.dma_start(
                        xh1[:, dc * DJ + dj, :], ys0[dc][dj * P : (dj + 1) * P, :]
                    )
            ysb1 = []
            for dc in range(NDC):
                y = ypool.tile([P, DJ, NTOK], f32, tag=f"ysb{dc}", name=f"ysb1_{dc}")
                nc.scalar.activation(
                    y[:], xh1[:, dc * DJ : (dc + 1) * DJ, :],
                    mybir.ActivationFunctionType.Copy, scale=alpha_sb[:, 0:1],
                )
                ysb1.append(y)

            ffn(1, xh1, wbc1, ysb1)

            # ---- tail: per-half ReduceScatter straight into yout
            yp1 = [
                dramp.tile([DCH, NTOK], bf16, tag=f"yp1_{dc}", name=f"yp1_{dc}")
                for dc in range(NDC)
            ]
            for dc in range(NDC):
                ybf = ybfp.tile([P, DJ, NTOK], bf16, tag=f"ybf{dc}")
                nc.vector.tensor_copy(ybf[:], ysb1[dc][:])
                for dj in range(DJ):
                    nc.sync.dma_start(
                        yp1[dc][dj * P : (dj + 1) * P, :], ybf[:, dj, :]
                    )
            HQ = DCH // 2  # 256 rows received per core per half
            yf = [
                dramp.tile([HQ, NTOK], bf16, tag=f"yf_{dc}", name=f"yf_{dc}")
                for dc in range(NDC)
            ]
            for dc in range(NDC):
                nc.gpsimd.collective_compute(
                    "ReduceScatter", mybir.AluOpType.add, replica_groups=PAIRS,
                    ins=[yp1[dc][:]], outs=[yf[dc][:]],
                )
            for dc in range(NDC):
                nc.sync.dma_start(yout.ap()[dc * HQ : (dc + 1) * HQ, :], yf[dc][:])
    return nc


_CACHE = {}


def _get_compiled():
    if "nc" not in _CACHE:
        nc = bacc.Bacc("TRN2", target_bir_lowering=False, debug=False, num_devices=8)
        build_moe(nc)
        nc.compile()
        _CACHE["nc"] = nc
    return _CACHE["nc"]


def kernel(x, protos, W1, W2, k):
    assert int(k) == 2
    B, S, Dx = x.shape
    L, E, D, F = W1.shape[0], W1.shape[1], W1.shape[2], W1.shape[3]
    N = B * S
    assert (B, S, Dx, L, E, D, F) == (2, 1024, 1024, 2, 8, 1024, 2048)
    NTOK = N // 4  # tokens per shard
    EG = E // 2    # experts per group

    nc = _get_compiled()

    xT = np.ascontiguousarray(np.asarray(x, dtype=np.float32).reshape(N, D).T)
    protT = np.ascontiguousarray(
        np.asarray(protos, dtype=np.float32).transpose(0, 2, 1)
    )
    W1b = np.asarray(W1, dtype=np.float32).astype(ml_dtypes.bfloat16)
    W2b = np.asarray(W2, dtype=np.float32).astype(ml_dtypes.bfloat16)

    in_maps = []
    for c in range(8):
        g, t = c // 4, c % 4
        alpha = np.full((P, 1), 1.0 if g == 0 else 0.0, dtype=np.float32)
        es = np.zeros((E, EG), dtype=np.float32)
        for e in range(EG):
            es[g * EG + e, e] = 1.0
        in_maps.append(
            {
                "xT": np.ascontiguousarray(xT[:, t * NTOK : (t + 1) * NTOK]),
                "xTh": np.ascontiguousarray(
                    xT[:, t * NTOK : (t + 1) * NTOK]
                ).astype(ml_dtypes.bfloat16),
                "prot": protT,
                "w1": np.ascontiguousarray(W1b[:, g * EG : (g + 1) * EG]),
                "w2": np.ascontiguousarray(W2b[:, g * EG : (g + 1) * EG]),
                "alpha": alpha,
                "esel": es,
            }
        )

    global _LAST_IN_MAPS
    _LAST_IN_MAPS = in_maps

    from concourse.bass_utils import run_bass_kernel_spmd

    res = run_bass_kernel_spmd(nc, in_maps, list(range(8)))
    # yout layout per core: rows [0:256] = its quarter of d-half 0,
    # rows [256:512] = its quarter of d-half 1.  Rank 0 of each pair (g=0)
    # receives the first quarter of each half.
    HQ = 256
    out_T = np.empty((D, N), dtype=np.float32)
    for t in range(4):
        lo = res.results[t]["yout"].astype(np.float32)
        hi = res.results[t + 4]["yout"].astype(np.float32)
        sl = slice(t * NTOK, (t + 1) * NTOK)
        out_T[0:256, sl] = lo[0:HQ]
        out_T[256:512, sl] = hi[0:HQ]
        out_T[512:768, sl] = lo[HQ:]
        out_T[768:1024, sl] = hi[HQ:]
    return np.ascontiguousarray(out_T.T).reshape(B, S, D).astype(np.float32)
